# revision 10
# baseline (speedup 1.0000x reference)
"""Multi-head self-attention (no mask) on 8 TRN2 NeuronCores.

Problem: B=2, T=2048, C=1024, H=16 heads, D=64.
    q/k/v = x @ W{q,k,v}.T + b;  att = softmax(q k^T / sqrt(D));
    y = att v;  out = y @ Wp.T + bp.

Sharding: core (b, g) with b in {0,1} batches x g in {0..3} head-groups of 4
heads.  Each core computes q/k/v for its 4 heads over the full sequence of its
batch, attention for those heads, and the partial output projection through its
256 columns of Wp.  The host sums the 4 partial projections per batch and adds
bp.  No device collectives needed.

v6 design (trace-driven):
  - All transposes on the HOST; every operand pre-tiled on the host into the
    exact SBUF layout (partition-major, chunk-major for x) so DMAs move one
    large contiguous segment per partition.  x chunk 0 is split across both
    HWDGE queues to cut the head stall.
  - ALL matmul operands bf16 (rel-err budget 2e-2, measured ~6e-3).
  - q/k/v projections all in one dense dependency-free PE stream (phase A)
    where FWL prefetch pipelines perfectly (~165ns/MM measured).
  - Attention runs one global software pipeline over all (tq, pi, s)
    iterations, paced by ACT exp (~1000ns/tile measured cadence); the P.V
    matmuls run L=4 iterations behind S/exp.  The S-pair uses PE row tiling
    (heads at rows 0-63/64-127, concurrent, 389ns/pair); the P.V stationary
    [1|v|0-pad] is padded to 128 columns so Fast Weight Load stays enabled
    (a 65-column stationary forced the slow LDWEIGHTS path).  Softmax
    denominators land on PSUM partition 0 (leading ones column) where the
    DVE reciprocal and gpsimd broadcast read them directly.
  - PSUM accumulators are drained to SBUF by the DVE right after the last
    P.V matmul, freeing banks for the next head pair without stalls.
  - The output projection for chunk tq-1 is fed one matmul per iteration
    into the PE's slack (gated 20 iterations into the next window so the
    normalize chain it reads has executed).  The final chunk processes
    pi=1 first and the tail accumulates j=1 before j=0, so the last
    normalize chain overlaps the tail's first matmuls.
"""

import sys
from collections import deque
from contextlib import ExitStack

import ml_dtypes
import numpy as np

if "/opt/trn_rl_repo" not in sys.path:
    sys.path.insert(0, "/opt/trn_rl_repo")

import concourse.bass as bass
import concourse.mybir as mybir
import concourse.tile as tile
from concourse import bacc
from concourse.bass_utils import run_bass_kernel_spmd

F32 = mybir.dt.float32
BF16 = mybir.dt.bfloat16
Act = mybir.ActivationFunctionType
BNP = ml_dtypes.bfloat16

P = 128
B, C, HEADS, D = 2, 1024, 16, 64
GROUPS = 4              # head groups (tensor-parallel dimension)
HLOC = HEADS // GROUPS  # 4 heads per core
G = HLOC * D            # 256 channels per core
KT = C // P             # 8 contraction tiles
VW = D + 1              # [1 | v] group width
TQ = 512                # query chunk (matmul moving free dim)


def build(T=2048):
    NTQ = T // TQ
    NS = T // P         # key tiles
    L = 4               # P.V lag (iterations) in the global pipeline

    nc = bacc.Bacc("TRN2", target_bir_lowering=False, debug=False)
    # host-pretiled inputs:
    #   xh  [NTQ, P, KT, TQ] : xh[c, p, a, t] = x[c*TQ+t, a*P+p]
    #   wh  [P, KT, G]       : wh[p, a, g]    = W[g, a*P+p]   (for q/k/v)
    #   wph [P, 2, C]        : wph[p, j, c]   = Wp[c, j*P+p]
    xh = nc.dram_tensor("xh", [NTQ, P, KT, TQ], BF16, kind="ExternalInput")
    wqh = nc.dram_tensor("wqh", [P, KT, G], BF16, kind="ExternalInput")
    wkh = nc.dram_tensor("wkh", [P, KT, G], BF16, kind="ExternalInput")
    wvh = nc.dram_tensor("wvh", [P, KT, G], BF16, kind="ExternalInput")
    wph = nc.dram_tensor("wph", [P, 2, C], BF16, kind="ExternalInput")
    bq = nc.dram_tensor("bq", [G], F32, kind="ExternalInput")
    bk = nc.dram_tensor("bk", [G], F32, kind="ExternalInput")
    bv = nc.dram_tensor("bv", [G], F32, kind="ExternalInput")
    out = nc.dram_tensor("out", [T, C], BF16, kind="ExternalOutput")

    with tile.TileContext(nc) as tc, ExitStack() as ctx:
        persist = ctx.enter_context(tc.tile_pool(name="persist", bufs=1))

        ones2 = persist.tile([P, HLOC, 1], BF16, tag="ones2")
        nc.gpsimd.memset(ones2[:], 1.0)

        bq_pp = persist.tile([P, 2], F32, tag="bq_pp")
        bk_pp = persist.tile([P, 2], F32, tag="bk_pp")
        bv_row = persist.tile([1, G], F32, tag="bv_row")
        bv_bc = persist.tile([P, G], F32, tag="bv_bc")

        x_sb = persist.tile([P, NTQ, KT, TQ], BF16, tag="x_sb")
        wq_sb = persist.tile([P, KT, G], BF16, tag="wq_sb")
        wk_sb = persist.tile([P, KT, G], BF16, tag="wk_sb")
        wv_sb = persist.tile([P, KT, G], BF16, tag="wv_sb")
        wp_sb = persist.tile([P, 2, C], BF16, tag="wp_sb")

        # [1 | v | zero-pad] per (key-tile, head): 128 stationary columns so
        # FWL stays on; P.V output rows 65..127 accumulate zeros, unread.
        v_sb = persist.tile([P, NS, HLOC, P], BF16, tag="v_sb")
        nc.gpsimd.memset(v_sb[:], 0.0)

        # first compute needs x chunk 0 (split across both queues) + Wk
        nc.scalar.dma_start(wk_sb[:], wkh[:, :, :])
        nc.sync.dma_start(x_sb[0:64, 0], xh[0, 0:64])
        nc.scalar.dma_start(x_sb[64:P, 0], xh[0, 64:P])
        nc.sync.dma_start(bq_pp[:], bq[:].rearrange("(m p) -> p m", p=P))
        nc.sync.dma_start(bk_pp[:], bk[:].rearrange("(m p) -> p m", p=P))
        nc.sync.dma_start(bv_row[:], bv[None, :])
        nc.gpsimd.partition_broadcast(bv_bc[:, :], bv_row[0:1, :], channels=P)
        nc.scalar.dma_start(wv_sb[:], wvh[:, :, :])
        for c in range(1, NTQ):
            nc.sync.dma_start(x_sb[:, c], xh[c])
        nc.scalar.dma_start(wq_sb[:], wqh[:, :, :])
        nc.scalar.dma_start(wp_sb[:], wph[:, :, :])

        qT = persist.tile([P, 2, T], BF16, tag="qT")
        kT = persist.tile([P, 2, T], BF16, tag="kT")
        yT = persist.tile([P, 2, T], BF16, tag="yT")

        # ---------------- phase A: all q/k/v projections ----------------
        with tc.tile_pool(name="pa", bufs=2, space="PSUM") as pa:
            for c in range(NTQ):
                cs = slice(c * TQ, (c + 1) * TQ)
                for m in range(2):
                    pk = pa.tile([P, TQ], F32, tag="pk")
                    for kk in range(KT):
                        nc.tensor.matmul(
                            pk[:],
                            wk_sb[:, kk, m * P : (m + 1) * P],
                            x_sb[:, c, kk, :],
                            start=(kk == 0),
                            stop=(kk == KT - 1),
                        )
                    nc.scalar.activation(
                        kT[:, m, cs], pk[:], Act.Identity,
                        bias=bk_pp[:, m : m + 1], scale=1.0,
                    )
                if c == 0:
                    for m in range(2):
                        pq = pa.tile([P, TQ], F32, tag="pk")
                        for kk in range(KT):
                            nc.tensor.matmul(
                                pq[:],
                                wq_sb[:, kk, m * P : (m + 1) * P],
                                x_sb[:, c, kk, :],
                                start=(kk == 0),
                                stop=(kk == KT - 1),
                            )
                        nc.scalar.activation(
                            qT[:, m, cs], pq[:], Act.Identity,
                            bias=bq_pp[:, m : m + 1], scale=1.0,
                        )
                for s in range(4 * c, 4 * c + 4):
                    si = s % 4
                    pv = pa.tile([P, G], F32, tag="pv")
                    for kk in range(KT):
                        nc.tensor.matmul(
                            pv[:],
                            x_sb[:, c, kk, si * P : (si + 1) * P],
                            wv_sb[:, kk, :],
                            start=(kk == 0),
                            stop=(kk == KT - 1),
                        )
                    vs = v_sb[:, s]
                    nc.vector.tensor_tensor(
                        vs[:, :, 1 : VW],
                        pv[:].rearrange("p (h d) -> p h d", d=D),
                        bv_bc[:].rearrange("p (h d) -> p h d", d=D),
                        op=mybir.AluOpType.add,
                    )
                    nc.vector.tensor_copy(vs[:, :, 0:1], ones2[:])

        # ---------------- phase B: pipelined attention + out-projection -----
        with (
            tc.tile_pool(name="ptp", bufs=L + 2) as ptp,
            tc.tile_pool(name="npool", bufs=2) as npool,
            tc.tile_pool(name="osb", bufs=2) as osb_pool,
            tc.tile_pool(name="sps", bufs=2, space="PSUM") as sps,
            tc.tile_pool(name="yps", bufs=1, space="PSUM") as yps,
            tc.tile_pool(name="xps", bufs=2, space="PSUM") as xps,
        ):
            def qnext_steps(tqn):
                """q^T projection for chunk tqn; ('pe'|'other', closure) steps."""
                tqs = slice(tqn * TQ, (tqn + 1) * TQ)
                for m in range(2):
                    pq = xps.tile([P, TQ], F32, tag="px")
                    for kk in range(KT):
                        yield "pe", lambda m=m, kk=kk, pq=pq: nc.tensor.matmul(
                            pq[:],
                            wq_sb[:, kk, m * P : (m + 1) * P],
                            x_sb[:, tqn, kk, :],
                            start=(kk == 0),
                            stop=(kk == KT - 1),
                        )
                    yield "other", lambda m=m, pq=pq: nc.vector.tensor_scalar_add(
                        qT[:, m, tqs], pq[:], bq_pp[:, m : m + 1]
                    )

            def oproj_steps(tqp, tail=False):
                """output projection for query chunk tqp (4 row-tiles).

                Tail chains accumulate j=1 before j=0: j=1 reads the head
                pair normalized early in the swapped last window, so the
                first tail matmuls overlap the final normalize chain.
                """
                jorder = (1, 0) if tail else (0, 1)
                for mi in range(4 * tqp, 4 * tqp + 4):
                    ob = osb_pool.tile([P, C], BF16, tag="ob")
                    for n in range(2):
                        po = xps.tile([P, 512], F32, tag="px")
                        for ji, j in enumerate(jorder):
                            yield "pe", lambda mi=mi, n=n, j=j, po=po, ji=ji: nc.tensor.matmul(
                                po[:],
                                yT[:, j, mi * P : (mi + 1) * P],
                                wp_sb[:, j, n * 512 : (n + 1) * 512],
                                start=(ji == 0),
                                stop=(ji == 1),
                            )
                        if tail and n == 0:
                            yield "other", lambda n=n, po=po, ob=ob: nc.scalar.activation(
                                ob[:, n * 512 : (n + 1) * 512], po[:], Act.Identity,
                                bias=0.0, scale=1.0,
                            )
                        else:
                            yield "other", lambda n=n, po=po, ob=ob: nc.vector.tensor_copy(
                                ob[:, n * 512 : (n + 1) * 512], po[:]
                            )
                    eng = nc.scalar if (tail and mi % 2) else nc.sync
                    yield "other", lambda mi=mi, ob=ob, eng=eng: eng.dma_start(
                        out[mi * P : (mi + 1) * P, :], ob[:]
                    )

            def normalize(pi, tq, srcs, tail=False):
                """softmax-normalize into yT.

                srcs[hh] rows: 0 = denominator, 1..64 = unnormalized y.  The
                DVE reciprocal and gpsimd broadcast read partition 0 directly;
                normalized rows reach their yT partitions via SBUF->SBUF DMA
                shift (DVE partition bases must be 32-aligned, so the
                multiply covers rows 0..64 and the DMA reads rows 1..64).
                """
                tqs = slice(tq * TQ, (tq + 1) * TQ)
                for hh in range(2):
                    src = srcs[hh]
                    recip0 = npool.tile([1, TQ], F32, tag=f"recip0{hh}",
                                        name=f"recip0{hh}")
                    nc.vector.reciprocal_approx_fast(recip0[0:1, :], src[0:1, :])
                    bcast = npool.tile([VW, TQ], F32, tag=f"bcast{hh}",
                                       name=f"bcast{hh}")
                    nc.gpsimd.partition_broadcast(
                        bcast[:, :], recip0[0:1, :], channels=VW
                    )
                    ytmp = npool.tile([VW, TQ], BF16, tag=f"ytmp{hh}",
                                      name=f"ytmp{hh}")
                    nc.vector.tensor_mul(
                        ytmp[0:VW, :], src[0:VW, :], bcast[0:VW, :]
                    )
                    eng = nc.scalar if tail else nc.sync
                    eng.dma_start(yT[hh * D : (hh + 1) * D, pi, tqs], ytmp[1:VW, :])

            def pump(extras, npe):
                while extras:
                    kind, fn = extras[0]
                    if kind == "pe":
                        if npe == 0:
                            return
                        npe -= 1
                    extras.popleft()
                    fn()

            iters = [
                (tq, pi, s)
                for tq in range(NTQ)
                for pi in ((1, 0) if tq == NTQ - 1 else (0, 1))
                for s in range(NS)
            ]
            NIT = len(iters)
            extras = deque()
            pending = []        # (ready_u, steps)
            py = [None, None]
            for u in range(NIT + L):
                if u < NIT:
                    tq, pi, s = iters[u]
                    if u % (2 * NS) == 0:
                        if tq + 1 < NTQ:
                            pending.append((u, deque(qnext_steps(tq + 1))))
                        if tq > 0:
                            # past the normalize(tq-1, pi=1) chain's execution
                            pending.append((u + 26, deque(oproj_steps(tq - 1))))
                    tqs = slice(tq * TQ, (tq + 1) * TQ)
                    sp = sps.tile([P, 2 * TQ], F32, tag="sp")
                    for hh in range(2):
                        bp_ = 64 * hh
                        nc.tensor.matmul(
                            sp[:, hh * TQ : (hh + 1) * TQ],
                            kT[bp_ : bp_ + 64, pi, s * P : (s + 1) * P],
                            qT[bp_ : bp_ + 64, pi, tqs],
                            start=True,
                            stop=True,
                        )
                    pt = ptp.tile([P, 2 * TQ], BF16, tag="pt")
                    nc.scalar.activation(
                        pt[:], sp[:], Act.Exp, scale=1.0 / np.sqrt(D)
                    )
                    iters[u] = (tq, pi, s, pt)
                if u >= L:
                    tq2, pi2, s2, pt2 = iters[u - L]
                    last = u - L == NIT - 1
                    if s2 == 0:
                        py[0] = yps.tile([P, TQ], F32, tag="py0", name="py0")
                        py[1] = yps.tile([P, TQ], F32, tag="py1", name="py1")
                    for hh in range(2):
                        h = 2 * pi2 + hh
                        nc.tensor.matmul(
                            py[hh][:],
                            v_sb[:, s2, h, :],
                            pt2[:, hh * TQ : (hh + 1) * TQ],
                            start=(s2 == 0),
                            stop=(s2 == NS - 1),
                        )
                    if s2 == NS - 1:
                        if last:
                            # no next head pair: normalize straight off PSUM
                            normalize(pi2, tq2, [py[0][0:VW], py[1][0:VW]],
                                      tail=True)
                        else:
                            # drain accumulators to SBUF, freeing the PSUM
                            # banks for the next head pair ~1 iter later
                            pys = [None, None]
                            for hh in range(2):
                                pys[hh] = npool.tile(
                                    [VW, TQ], F32,
                                    tag=f"pys{hh}", name=f"pys{hh}",
                                )
                                nc.vector.tensor_copy(pys[hh][:], py[hh][0:VW, :])
                            normalize(pi2, tq2, pys)
                while pending and pending[0][0] <= u:
                    extras.extend(pending.pop(0)[1])
                npe_left = sum(1 for k, _ in extras if k == "pe")
                slots_left = max(1, (2 * NS - (u % (2 * NS))))
                pump(extras, 2 if npe_left > slots_left else 1)
            pump(extras, 1 << 30)
            # output projection for the final chunk
            tail = deque(oproj_steps(NTQ - 1, tail=True))
            pump(tail, 1 << 30)

    nc.finalize()
    return nc


_NC_CACHE = {}


def _get_nc(T=2048):
    if T not in _NC_CACHE:
        _NC_CACHE[T] = build(T=T)
    return _NC_CACHE[T]


def _sbufify_w(W_slice_T):
    """[C, G] -> [P, KT, G] with wh[p, a, g] = W^T[a*P+p, g]."""
    return np.ascontiguousarray(
        W_slice_T.reshape(KT, P, -1).transpose(1, 0, 2)
    ).astype(BNP)


def _make_in_maps(x, Wq, bq, Wk, bk, Wv, bv, Wp):
    in_maps = []
    wqhs = [_sbufify_w(Wq[g * G : (g + 1) * G, :].T) for g in range(GROUPS)]
    wkhs = [_sbufify_w(Wk[g * G : (g + 1) * G, :].T) for g in range(GROUPS)]
    wvhs = [_sbufify_w(Wv[g * G : (g + 1) * G, :].T) for g in range(GROUPS)]
    # wph[p, j, c] = Wp[c, g*G + j*P + p]
    wphs = [
        np.ascontiguousarray(
            Wp[:, g * G : (g + 1) * G].T.reshape(2, P, C).transpose(1, 0, 2)
        ).astype(BNP)
        for g in range(GROUPS)
    ]
    for b in range(B):
        T = x.shape[1]
        # xh[c, p, a, t] = x[b][c*TQ+t, a*P+p]
        xh_b = np.ascontiguousarray(
            x[b].T.reshape(KT, P, T // TQ, TQ).transpose(2, 1, 0, 3)
        ).astype(BNP)
        for g in range(GROUPS):
            sl = slice(g * G, (g + 1) * G)
            in_maps.append(
                {
                    "xh": xh_b,
                    "wqh": wqhs[g],
                    "wkh": wkhs[g],
                    "wvh": wvhs[g],
                    "wph": wphs[g],
                    "bq": np.ascontiguousarray(bq[sl], dtype=np.float32),
                    "bk": np.ascontiguousarray(bk[sl], dtype=np.float32),
                    "bv": np.ascontiguousarray(bv[sl], dtype=np.float32),
                }
            )
    return in_maps


def run(inputs, trace=False):
    """Run on 8 cores; returns (out [B,T,C] fp32, BassKernelResults)."""
    x = np.asarray(inputs["x"], dtype=np.float32)
    T = x.shape[1]
    in_maps = _make_in_maps(
        x,
        np.asarray(inputs["Wq"]), np.asarray(inputs["bq"]),
        np.asarray(inputs["Wk"]), np.asarray(inputs["bk"]),
        np.asarray(inputs["Wv"]), np.asarray(inputs["bv"]),
        np.asarray(inputs["Wp"]),
    )
    nc = _get_nc(T)
    res = run_bass_kernel_spmd(
        nc, in_maps, core_ids=list(range(B * GROUPS)), trace=trace
    )
    bp = np.asarray(inputs["bp"], dtype=np.float32)
    parts = [
        np.asarray(res.results[i]["out"], dtype=np.float32)
        for i in range(B * GROUPS)
    ]
    out = np.stack(
        [sum(parts[b * GROUPS : (b + 1) * GROUPS]) for b in range(B)]
    ) + bp[None, None, :]
    return out.astype(np.float32), res


def kernel(**inputs):
    out, _ = run(inputs, trace=False)
    return out
